# revision 15
# baseline (speedup 1.0000x reference)
"""2-layer GAT (PyG GATConv semantics) on 8 Trainium2 NeuronCores via Bass/Tile.

Contract: kernel(**inputs) takes the FULL inputs of reference.setup_inputs()
and returns the FULL [16, 4096, 128] float32 output.

v2 design (dst-node sharding, degree-sorted blocks, dma_gather edge fetch):
- Core c owns dst nodes [c*8192, (c+1)*8192). Within a core, nodes are ranked
  by in-degree (self-loops excluded; they are folded analytically in the
  epilogue). Block b = ranks [128b, 128b+128); partition p holds the block's
  p-th node. Slot (b, p, j) = j-th in-edge of that node, padded per block to
  K[b] = max cross-core block degree (degree sorting makes padding ~5%).
- Node tables in DRAM, bf16, ONE physical layout shared by both layers
  (slice-major rank order), so a single int16 index array (phys(src)-32768,
  signed, table base mid-table) and a single pad mask drive both layers:
    t1 [N,128]: [h1(64) | asrc1(8) | adst1(8) | pad]     (256B rows)
    t2 [N,256]: [h2'(128) | asrc2'(1) | pad]             (512B rows)
- Phase A (sharded): t1 shard = xT @ [W1|wsrc1|wdst1], AllGather in 8 slices.
- Edge phase per block: one dma_gather (queue b%4, ~128*K[b] rows); softmax
  weights w = exp(lrelu(asrc[src]+adst[dst]+mask)) batched per block on
  DVE+ACT (denominator via ACT accum_out / reduce); rhs = gat*w in one fused
  DVE op; PSUM accumulation via identity-lhsT matmuls (one per 128-edge
  chunk). Self-loop terms w_self*h_self are added in the epilogue.
- L1 epilogue: y = acc/s + b1; (elu+1) fold: t2 stores h2' = (elu+1)@W2ext
  with W2ext = [W2 | W2@a2s | W2@a2d]; bias/logit constants folded into
  b2eff = b2 - colsum(W2) and adw2 = adst2' - c2. t2s rows AllGathered into
  t2 after every 8 blocks.
- L2 epilogue: out = acc2/s2 + b2eff, rows in rank order; host unpermutes.
"""

import os
import sys

import numpy as np

if "/opt/trn_rl_repo" not in sys.path:
    sys.path.insert(0, "/opt/trn_rl_repo")

import concourse.bass as bass
import concourse.bacc as bacc
import concourse.mybir as mybir
import concourse.tile as tile
from concourse.tile_rust import add_dep_helper

F32 = mybir.dt.float32
BF16 = mybir.dt.bfloat16
I16 = mybir.dt.int16
AOP = mybir.AluOpType
ACT = mybir.ActivationFunctionType

NEG_SLOPE = 0.2
NCORES = 8
BLK = 128
NSH = 8192
NBLK = NSH // BLK
NSLICE = 8
BPS = NBLK // NSLICE          # blocks per AG slice
SLN = NSH // NSLICE           # own rows per AG slice
T1W = 128                     # t1 cols (bf16): h1 64 | asrc 8 | adst 8 | pad
T2W = 256                     # t2 cols (bf16): h2' 128 | asrc2' 1 | pad
MASKVAL = -1e30
SCRATCH = 64 * 1024
USE_4D = os.environ.get("K4D", "1") == "1"


class Cfg:
    def __init__(self, n_nodes, d_in, h1, c1, d2, kprof):
        self.N = n_nodes
        self.D = d_in
        self.H1 = h1
        self.C1 = c1
        self.D1 = h1 * c1
        self.D2 = d2
        self.KPROF = list(kprof)          # per-block chunk counts (uniform)
        self.KSUM = int(sum(kprof))
        self.KOFF = np.concatenate([[0], np.cumsum(kprof)]).astype(int)


# ---------------------------------------------------------------------------
# host-side schedule
# ---------------------------------------------------------------------------
def _schedule(src, dst, N):
    """Degree-ranked per-core blocks; shared slot arrays for both layers.

    Returns (kprof, perm[c], idxw[c], maskw[c]) where idxw is the wrapped,
    replicated int16 index array [128, 8*KSUM] (values phys(src)-32768) and
    maskw the pad mask [128, KSUM] float32 (0 valid / MASKVAL pad).
    """
    core = dst >> 13
    perms = []
    degs = np.zeros((NCORES, NSH), dtype=np.int64)
    for c in range(NCORES):
        cnt = np.bincount(dst[core == c] - c * NSH, minlength=NSH)
        rank_to_node = np.argsort(-cnt, kind="stable")
        perms.append(rank_to_node)
        degs[c] = cnt[rank_to_node]
    # uniform per-block K profile (max over cores of block max degree)
    kprof = []
    for b in range(NBLK):
        kprof.append(int(max(1, degs[:, b * BLK:(b + 1) * BLK].max())))
    kprof = np.asarray(kprof, dtype=np.int64)
    ksum = int(kprof.sum())

    # phys mapping: node -> slice-major rank position (same for t1/t2)
    node_to_rank = np.zeros(N, dtype=np.int64)
    for c in range(NCORES):
        node_to_rank[perms[c] + c * NSH] = np.arange(NSH)
    s_of = node_to_rank >> 10
    phys = s_of * NSH + (np.arange(N, dtype=np.int64) >> 13) * 1024 \
        + (node_to_rank & 1023)

    order = np.argsort(dst, kind="stable")
    s_sorted, d_sorted = src[order], dst[order]
    starts = np.zeros(N + 1, dtype=np.int64)
    np.cumsum(np.bincount(d_sorted, minlength=N), out=starts[1:])

    # The gather ucode trims TRAILING NEGATIVE indices from each list, so the
    # last linear slot (p=127, j=kb-1) of every block must be >= 0. If node
    # 127's list is full and entirely negative, widen that block by one pad.
    for _ in range(3):
        koff = np.concatenate([[0], np.cumsum(kprof)])
        bump = np.zeros(NBLK, dtype=bool)
        for c in range(NCORES):
            for b in range(NBLK):
                kb = int(kprof[b])
                n = perms[c][b * BLK + 127] + c * NSH
                deg = int(starts[n + 1] - starts[n])
                if deg >= kb:
                    vals = phys[s_sorted[starts[n]:starts[n] + kb]] - 32768
                    if (vals < 0).all():
                        bump[b] = True
        if not bump.any():
            break
        kprof = kprof + bump.astype(np.int64)
    ksum = int(kprof.sum())
    koff = np.concatenate([[0], np.cumsum(kprof)])

    idx_all, mask_all = [], []
    for c in range(NCORES):
        lin = np.zeros((ksum, BLK), dtype=np.int16)      # [slotcol, p]
        msk = np.zeros((BLK, ksum), dtype=np.float32)
        msk[:] = MASKVAL
        for b in range(NBLK):
            kb = int(kprof[b])
            for p in range(BLK):
                n = perms[c][b * BLK + p] + c * NSH
                e0, e1 = int(starts[n]), int(starts[n + 1])
                deg = e1 - e0
                if deg:
                    vals = (phys[s_sorted[e0:e1]] - 32768).astype(np.int16)
                    if p == 127 and deg >= kb and vals[kb - 1] < 0:
                        nn = np.where(vals[:kb] >= 0)[0]
                        assert len(nn), "unfixable trailing-negative block"
                        vals = vals.copy()
                        vals[nn[0]], vals[kb - 1] = vals[kb - 1], vals[nn[0]]
                    lin[koff[b]:koff[b] + deg, p] = vals
                    msk[p, koff[b]:koff[b] + deg] = 0.0
        # wrap: linear i = j*128+p within each block -> [16, 8*K] per block
        iw = np.zeros((16, 8 * ksum), dtype=np.int16)
        for b in range(NBLK):
            kb = int(kprof[b])
            seg = lin[koff[b]:koff[b] + kb, :].reshape(-1)  # i = j*128+p
            ii = np.arange(kb * BLK)
            iw[ii % 16, 8 * koff[b] + ii // 16] = seg
        idx_all.append(np.tile(iw, (8, 1)))
        mask_all.append(msk)
    return kprof, perms, idx_all, mask_all


# ---------------------------------------------------------------------------
# device program
# ---------------------------------------------------------------------------
def build_program(cfg, c2_const):
    D, H1, C1, D1, D2 = cfg.D, cfg.H1, cfg.C1, cfg.D1, cfg.D2
    KPROF, KOFF, KSUM = cfg.KPROF, cfg.KOFF, cfg.KSUM
    KMAX = max(KPROF)
    N = cfg.N

    nc = bacc.Bacc("TRN2", target_bir_lowering=False, debug=False,
                   num_devices=NCORES, num_swdge_queues=4,
                   dynamic_dma_scratch_size=SCRATCH)

    xt = nc.dram_tensor("xt", [D, NSH], F32, kind="ExternalInput")
    wpack1 = nc.dram_tensor("wpack1", [D, D1 + 2 * H1], F32, kind="ExternalInput")
    w2ext = nc.dram_tensor("w2ext", [D1, D2 + 2], F32, kind="ExternalInput")
    b1r = nc.dram_tensor("b1r", [128, D1], F32, kind="ExternalInput")
    b2effr = nc.dram_tensor("b2effr", [128, D2], F32, kind="ExternalInput")
    iota = nc.dram_tensor("iota", [128, 128], F32, kind="ExternalInput")
    iotac = nc.dram_tensor("iotac", [128, 1], F32, kind="ExternalInput")
    eidx = nc.dram_tensor("eidx", [128, 8 * KSUM], I16, kind="ExternalInput")
    emask = nc.dram_tensor("emask", [128, KSUM], F32, kind="ExternalInput")
    out = nc.dram_tensor("out", [NSH, D2], F32, kind="ExternalOutput")

    dump = os.environ.get("KDUMP", "") == "1"
    t1s = nc.dram_tensor("t1s", [NSH, T1W], BF16, kind="Internal")
    t2s = nc.dram_tensor("t2s", [NSH, T2W], BF16, kind="Internal")
    if dump:
        t1d = nc.dram_tensor("t1d", [NSH, 80], F32, kind="ExternalOutput")
        t2d = nc.dram_tensor("t2d", [NSH, 130], F32, kind="ExternalOutput")
        yd = nc.dram_tensor("yd", [NSH, D1], F32, kind="ExternalOutput")
        sd = nc.dram_tensor("sd", [128, NBLK * H1], F32, kind="ExternalOutput")
        gd = nc.dram_tensor("gd", [128, KMAX * T1W], F32, kind="ExternalOutput")
    t1 = nc.dram_tensor("t1", [N, T1W], BF16, kind="Internal", addr_space="Shared")
    t2 = nc.dram_tensor("t2", [N, T2W], BF16, kind="Internal", addr_space="Shared")

    from concourse import library_config

    with tile.TileContext(nc) as tc:
        with tc.tile_pool(name="const", bufs=1) as cp:
            nc.gpsimd.load_library(library_config.mlp)
            con = {}
            for name, hndl in [("wpack1", wpack1), ("w2ext", w2ext),
                               ("b1r", b1r), ("b2effr", b2effr),
                               ("iota", iota), ("iotac", iotac)]:
                t = cp.tile(list(hndl.shape), hndl.dtype, tag=name)
                nc.sync.dma_start(out=t[:], in_=hndl[:])
                con[name] = t
            ident_bf = cp.tile([128, 128], BF16)
            nc.vector.tensor_tensor(
                out=ident_bf[:], in0=con["iotac"][:].to_broadcast([128, 128]),
                in1=con["iota"][:], op=AOP.is_equal)
            ident_f = cp.tile([128, 128], F32)
            nc.vector.tensor_tensor(
                out=ident_f[:], in0=con["iotac"][:].to_broadcast([128, 128]),
                in1=con["iota"][:], op=AOP.is_equal)
            con["ident_bf"] = ident_bf
            con["ident_f"] = ident_f
            # resident edge schedule
            eidx_t = cp.tile([128, 8 * KSUM], I16, tag="eidx")
            nc.sync.dma_start(out=eidx_t[:], in_=eidx[:])
            emask_t = cp.tile([128, KSUM], F32, tag="emask")
            nc.sync.dma_start(out=emask_t[:], in_=emask[:])
            con["eidx"] = eidx_t
            con["emask"] = emask_t

            # ---------------- phase A: t1 shard + AllGather ----------------
            ag1 = []
            with (tc.tile_pool(name="pa_ps", bufs=4, space="PSUM") as pps,
                  tc.tile_pool(name="pa_st", bufs=4) as pst):
                for t in range(NBLK):
                    ps = pps.tile([128, D1 + 2 * H1], F32, tag="ps")
                    xtile = pst.tile([128, 128], F32, tag="xtile")
                    nc.sync.dma_start(out=xtile[:], in_=xt[:, t * 128:(t + 1) * 128])
                    nc.tensor.matmul(out=ps[:], lhsT=xtile[:], rhs=con["wpack1"][:],
                                     start=True, stop=True)
                    stg = pst.tile([128, D1 + 2 * H1], BF16, tag="stg")
                    nc.scalar.copy(out=stg[:], in_=ps[:])
                    nc.sync.dma_start(
                        out=t1s[t * 128:(t + 1) * 128, 0:D1 + 2 * H1], in_=stg[:])
                    if dump:
                        stgf = pst.tile([128, D1 + 2 * H1], F32, tag="stgf")
                        nc.vector.tensor_copy(out=stgf[:], in_=stg[:])
                        nc.sync.dma_start(
                            out=t1d[t * 128:(t + 1) * 128, :], in_=stgf[:])
                    if (t + 1) % BPS == 0:
                        s = (t + 1) // BPS - 1
                        g = nc.gpsimd.collective_compute(
                            "AllGather", AOP.bypass,
                            replica_groups=[list(range(NCORES))],
                            ins=[t1s[s * SLN:(s + 1) * SLN, :]],
                            outs=[t1[s * SLN * NCORES:(s + 1) * SLN * NCORES, :]])
                        ag1.append(g)

            # own-shard L1 self data: [128p, NBLK, 80] (h1|asrc|adst)
            t1self = cp.tile([128, NBLK, D1 + 2 * H1], BF16, tag="t1self")
            src_ap = bass.AP(
                t1s[:].tensor, 0,
                [[T1W, 128], [BLK * T1W, NBLK], [1, D1 + 2 * H1]])
            nc.sync.dma_start(out=t1self[:], in_=src_ap)

            # batched L1 self weights: wself [128, NBLK*H1] f32
            wself1 = cp.tile([128, NBLK * H1], F32, tag="wself1")
            zs = cp.tile([128, NBLK * H1], F32, tag="zs")
            nc.vector.tensor_tensor(
                out=zs[:], in0=t1self[:, :, D1:D1 + H1],
                in1=t1self[:, :, D1 + H1:D1 + 2 * H1], op=AOP.add)
            nc.vector.scalar_tensor_tensor(
                out=zs[:], in0=zs[:], scalar=NEG_SLOPE, in1=zs[:],
                op0=AOP.mult, op1=AOP.max)
            nc.scalar.activation(out=wself1[:], in_=zs[:], func=ACT.Exp)

            adw2_all = cp.tile([128, NBLK], F32, tag="adw2")
            asrc2s_all = cp.tile([128, NBLK], F32, tag="asrc2s")

            # ---------------- L1 edge phase -------------------------------
            ag2 = []
            with (tc.tile_pool(name="e1_g", bufs=3) as pg,
                  tc.tile_pool(name="e1_w", bufs=2) as pw,
                  tc.tile_pool(name="e1_r", bufs=2) as pr,
                  tc.tile_pool(name="e1_acc", bufs=2, space="PSUM") as pacc,
                  tc.tile_pool(name="e1_tp", bufs=2, space="PSUM") as ptp,
                  tc.tile_pool(name="e1_h2", bufs=2, space="PSUM") as ph2,
                  tc.tile_pool(name="e1_ep", bufs=2) as pep):
                for b in range(NBLK):
                    kb = KPROF[b]
                    gat = pg.tile([128, KMAX, T1W], BF16, tag="gat")
                    gi = nc.gpsimd.dma_gather(
                        out_ap=gat[:, 0:kb, :], in_ap=t1[32768:, :],
                        idxs_ap=con["eidx"][:, 8 * KOFF[b]:8 * KOFF[b + 1]],
                        num_idxs=128 * kb, num_idxs_reg=128 * kb,
                        elem_size=T1W, queue_num=b % 4, single_packet=False)
                    for g in ag1:
                        add_dep_helper(gi.ins, g.ins, reason="t1 full-table read")

                    # w batch [128, kb*H1]
                    lg = pw.tile([128, KMAX * H1], F32, tag="lg")
                    adw = t1self[:, b:b + 1, D1 + H1:D1 + 2 * H1]
                    adw_bc = bass.AP(adw.tensor, adw.offset,
                                     [list(adw.ap[0]), [0, kb], [1, H1]])
                    nc.vector.tensor_tensor(
                        out=lg[:, 0:kb * H1].rearrange("p (k h) -> p k h", h=H1),
                        in0=gat[:, 0:kb, D1:D1 + H1], in1=adw_bc, op=AOP.add)
                    mslice = con["emask"][:, KOFF[b]:KOFF[b + 1]]
                    m_bc = bass.AP(mslice.tensor, mslice.offset,
                                   [list(mslice.ap[0]), list(mslice.ap[1]), [0, H1]])
                    nc.vector.tensor_tensor(
                        out=lg[:, 0:kb * H1].rearrange("p (k h) -> p k h", h=H1),
                        in0=lg[:, 0:kb * H1].rearrange("p (k h) -> p k h", h=H1),
                        in1=m_bc, op=AOP.add)
                    nc.vector.scalar_tensor_tensor(
                        out=lg[:, 0:kb * H1], in0=lg[:, 0:kb * H1],
                        scalar=NEG_SLOPE, in1=lg[:, 0:kb * H1],
                        op0=AOP.mult, op1=AOP.max)
                    w_t = pw.tile([128, KMAX * H1], F32, tag="w")
                    nc.scalar.activation(out=w_t[:, 0:kb * H1],
                                         in_=lg[:, 0:kb * H1], func=ACT.Exp)

                    # s[p,h] = sum_j w + wself
                    s_t = pep.tile([128, H1], F32, tag="s")
                    nc.vector.reduce_sum(
                        out=s_t[:],
                        in_=w_t[:, 0:kb * H1].rearrange("p (k h) -> p h k", h=H1),
                        axis=mybir.AxisListType.X)
                    nc.vector.tensor_add(
                        out=s_t[:], in0=s_t[:],
                        in1=wself1[:, b * H1:(b + 1) * H1])

                    # rhs = gat[:, :, 0:64] * w  (4D broadcast)
                    rhs = pr.tile([128, KMAX * D1], BF16, tag="rhs")
                    if USE_4D:
                        gv = gat[:, 0:kb, 0:D1].rearrange(
                            "p k (h c) -> p k h c", h=H1)
                        wv = bass.AP(
                            w_t[:].tensor, w_t[:].offset,
                            [list(w_t[:].ap[0]), [H1, kb], [1, H1], [0, C1]])
                        rv = rhs[:, 0:kb * D1].rearrange(
                            "p (k h c) -> p k h c", h=H1, c=C1)
                        nc.vector.tensor_tensor(out=rv, in0=gv, in1=wv, op=AOP.mult)
                    else:
                        for h in range(H1):
                            wcol = bass.AP(
                                w_t[:].tensor, w_t[:].offset + h,
                                [list(w_t[:].ap[0]), [H1, kb], [0, C1]])
                            nc.vector.tensor_tensor(
                                out=rhs[:, 0:kb * D1].rearrange(
                                    "p (k c) -> p k c", c=D1)[:, :, h * C1:(h + 1) * C1],
                                in0=gat[:, 0:kb, h * C1:(h + 1) * C1],
                                in1=wcol, op=AOP.mult)

                    acc = pacc.tile([128, D1], F32, tag="acc")
                    for j in range(kb):
                        nc.tensor.matmul(
                            out=acc[:], lhsT=con["ident_bf"][:],
                            rhs=rhs[:, j * D1:(j + 1) * D1],
                            start=(j == 0), stop=(j == kb - 1))

                    # ---- epilogue ----
                    vv = pep.tile([128, D1], F32, tag="vv")
                    wssl = wself1[:, b * H1:(b + 1) * H1]
                    wsv = bass.AP(wssl.tensor, wssl.offset,
                                  [list(wssl.ap[0]), [1, H1], [0, C1]])
                    h1self = t1self[:, b:b + 1, 0:D1]
                    h1v = bass.AP(h1self.tensor, h1self.offset,
                                  [list(h1self.ap[0]), [C1, H1], [1, C1]])
                    nc.vector.tensor_tensor(
                        out=vv[:].rearrange("p (h c) -> p h c", h=H1),
                        in0=h1v, in1=wsv, op=AOP.mult)
                    nc.vector.tensor_add(out=vv[:], in0=vv[:], in1=acc[:])
                    sinv = pep.tile([128, H1], F32, tag="sinv")
                    nc.vector.reciprocal(out=sinv[:], in_=s_t[:])
                    y = pep.tile([128, D1], F32, tag="y")
                    sinv_bc = bass.AP(
                        sinv[:].tensor, sinv[:].offset,
                        [list(sinv[:].ap[0]), [1, H1], [0, C1]])
                    nc.vector.tensor_tensor(
                        out=y[:].rearrange("p (h c) -> p h c", h=H1),
                        in0=vv[:].rearrange("p (h c) -> p h c", h=H1),
                        in1=sinv_bc, op=AOP.mult)
                    nc.vector.tensor_add(out=y[:], in0=y[:], in1=con["b1r"][:])
                    if dump:
                        nc.sync.dma_start(out=yd[b * BLK:(b + 1) * BLK, :],
                                          in_=y[:])
                        nc.sync.dma_start(out=sd[:, b * H1:(b + 1) * H1],
                                          in_=s_t[:])
                        if b == 0:
                            gdf = pep.tile([128, KMAX * T1W], F32, tag="gdf")
                            nc.vector.tensor_copy(
                                out=gdf[:],
                                in_=gat[:].rearrange("p k w -> p (k w)"))
                            nc.sync.dma_start(out=gd[:], in_=gdf[:])
                    tmin = pep.tile([128, D1], F32, tag="tmin")
                    nc.vector.tensor_scalar_min(out=tmin[:], in0=y[:], scalar1=0.0)
                    e_t = pep.tile([128, D1], F32, tag="e")
                    nc.scalar.activation(out=e_t[:], in_=tmin[:], func=ACT.Exp)
                    helu = pep.tile([128, D1], F32, tag="helu")
                    nc.vector.scalar_tensor_tensor(
                        out=helu[:], in0=y[:], scalar=0.0, in1=e_t[:],
                        op0=AOP.max, op1=AOP.add)
                    nc.vector.tensor_scalar_add(out=helu[:], in0=helu[:],
                                                scalar1=-1.0)
                    htp = ptp.tile([D1, 128], F32, tag="htp")
                    nc.tensor.transpose(out=htp[:], in_=helu[:],
                                        identity=con["ident_f"][:])
                    hts = pep.tile([D1, 128], F32, tag="hts")
                    nc.vector.tensor_copy(out=hts[:], in_=htp[:])
                    h2p = ph2.tile([128, D2 + 2], F32, tag="h2p")
                    nc.tensor.matmul(out=h2p[:], lhsT=hts[:], rhs=con["w2ext"][:],
                                     start=True, stop=True)
                    # stash asrc2', adw2 = adst2' - c2
                    nc.scalar.activation(
                        out=asrc2s_all[:, b:b + 1], in_=h2p[:, D2:D2 + 1],
                        func=ACT.Copy)
                    nc.scalar.activation(
                        out=adw2_all[:, b:b + 1], in_=h2p[:, D2 + 1:D2 + 2],
                        func=ACT.Copy)
                    stg2 = pep.tile([128, D2 + 1], BF16, tag="stg2")
                    nc.vector.tensor_copy(out=stg2[:], in_=h2p[:, 0:D2 + 1])
                    nc.sync.dma_start(
                        out=t2s[b * BLK:(b + 1) * BLK, 0:D2 + 1], in_=stg2[:])
                    if dump:
                        stg2f = pep.tile([128, D2 + 2], F32, tag="stg2f")
                        nc.vector.tensor_copy(out=stg2f[:], in_=h2p[:])
                        nc.sync.dma_start(
                            out=t2d[b * BLK:(b + 1) * BLK, 0:D2 + 1],
                            in_=stg2f[:, 0:D2 + 1])

                    if (b + 1) % BPS == 0:
                        s = (b + 1) // BPS - 1
                        g = nc.gpsimd.collective_compute(
                            "AllGather", AOP.bypass,
                            replica_groups=[list(range(NCORES))],
                            ins=[t2s[s * SLN:(s + 1) * SLN, :]],
                            outs=[t2[s * SLN * NCORES:(s + 1) * SLN * NCORES, :]])
                        ag2.append(g)

            # own-shard L2 self data
            t2self = cp.tile([128, NBLK, D2], BF16, tag="t2self")
            src2_ap = bass.AP(
                t2s[:].tensor, 0, [[T2W, 128], [BLK * T2W, NBLK], [1, D2]])
            d2s = nc.sync.dma_start(out=t2self[:], in_=src2_ap)
            wself2 = cp.tile([128, NBLK], F32, tag="wself2")
            zs2 = cp.tile([128, NBLK], F32, tag="zs2")
            nc.vector.tensor_tensor(out=zs2[:], in0=asrc2s_all[:],
                                    in1=adw2_all[:], op=AOP.add)
            nc.vector.scalar_tensor_tensor(
                out=zs2[:], in0=zs2[:], scalar=NEG_SLOPE, in1=zs2[:],
                op0=AOP.mult, op1=AOP.max)
            nc.scalar.activation(out=wself2[:], in_=zs2[:], func=ACT.Exp)

            # ---------------- L2 edge phase -------------------------------
            with (tc.tile_pool(name="e2_g", bufs=3) as pg2,
                  tc.tile_pool(name="e2_w", bufs=2) as pw2,
                  tc.tile_pool(name="e2_r", bufs=2) as pr2,
                  tc.tile_pool(name="e2_acc", bufs=2, space="PSUM") as pacc2,
                  tc.tile_pool(name="e2_ep", bufs=2) as pep2):
                for b in range(NBLK):
                    kb = KPROF[b]
                    gat2 = pg2.tile([128, KMAX, T2W], BF16, tag="gat2")
                    gi = nc.gpsimd.dma_gather(
                        out_ap=gat2[:, 0:kb, :], in_ap=t2[32768:, :],
                        idxs_ap=con["eidx"][:, 8 * KOFF[b]:8 * KOFF[b + 1]],
                        num_idxs=128 * kb, num_idxs_reg=128 * kb,
                        elem_size=T2W, queue_num=b % 4, single_packet=False)
                    for g in ag2:
                        add_dep_helper(gi.ins, g.ins, reason="t2 full-table read")

                    lg2 = pw2.tile([128, KMAX], F32, tag="lg2")
                    asr = gat2[:, 0:kb, D2:D2 + 1]
                    asr2d = bass.AP(asr.tensor, asr.offset,
                                    [list(asr.ap[0]), [T2W, kb]])
                    nc.vector.tensor_tensor(
                        out=lg2[:, 0:kb], in0=asr2d,
                        in1=adw2_all[:, b:b + 1].to_broadcast([128, kb]),
                        op=AOP.add)
                    nc.vector.tensor_add(
                        out=lg2[:, 0:kb], in0=lg2[:, 0:kb],
                        in1=con["emask"][:, KOFF[b]:KOFF[b + 1]])
                    nc.vector.scalar_tensor_tensor(
                        out=lg2[:, 0:kb], in0=lg2[:, 0:kb], scalar=NEG_SLOPE,
                        in1=lg2[:, 0:kb], op0=AOP.mult, op1=AOP.max)
                    w2t = pw2.tile([128, KMAX], F32, tag="w2t")
                    s2 = pep2.tile([128, 1], F32, tag="s2")
                    nc.scalar.activation(out=w2t[:, 0:kb], in_=lg2[:, 0:kb],
                                         func=ACT.Exp, accum_out=s2[:])
                    nc.vector.tensor_add(out=s2[:], in0=s2[:],
                                         in1=wself2[:, b:b + 1])

                    rhs2 = pr2.tile([128, KMAX * D2], BF16, tag="rhs2")
                    wv = bass.AP(w2t[:].tensor, w2t[:].offset,
                                 [list(w2t[:].ap[0]), [1, kb], [0, D2]])
                    nc.vector.tensor_tensor(
                        out=rhs2[:, 0:kb * D2].rearrange("p (k d) -> p k d", d=D2),
                        in0=gat2[:, 0:kb, 0:D2], in1=wv, op=AOP.mult)

                    acc2 = pacc2.tile([128, D2], F32, tag="acc2")
                    for j in range(kb):
                        nc.tensor.matmul(
                            out=acc2[:], lhsT=con["ident_bf"][:],
                            rhs=rhs2[:, j * D2:(j + 1) * D2],
                            start=(j == 0), stop=(j == kb - 1))

                    # epilogue: out = (acc2 + wself2*h2self)/s2 + b2eff
                    av = pep2.tile([128, D2], F32, tag="av")
                    ws_bc = wself2[:, b:b + 1].to_broadcast([128, D2])
                    nc.vector.tensor_tensor(out=av[:], in0=t2self[:, b, :],
                                            in1=ws_bc, op=AOP.mult)
                    nc.vector.tensor_add(out=av[:], in0=av[:], in1=acc2[:])
                    sinv2 = pep2.tile([128, 1], F32, tag="sinv2")
                    nc.vector.reciprocal(out=sinv2[:], in_=s2[:])
                    o1 = pep2.tile([128, D2], F32, tag="o1")
                    nc.scalar.activation(out=o1[:], in_=av[:], func=ACT.Copy,
                                         scale=sinv2[:])
                    o2 = pep2.tile([128, D2], F32, tag="o2")
                    nc.vector.tensor_add(out=o2[:], in0=o1[:], in1=con["b2effr"][:])
                    nc.sync.dma_start(out=out[b * BLK:(b + 1) * BLK, :], in_=o2[:])

    nc.compile()
    return nc


# ---------------------------------------------------------------------------
# host glue
# ---------------------------------------------------------------------------
def prepare(x, seq, edges, W1, att_src1, att_dst1, b1, W2, att_src2,
            att_dst2, b2):
    import ml_dtypes

    nb, ncn, d = x.shape
    N = nb * ncn
    H1, C1 = att_src1.shape
    D1 = H1 * C1
    D2 = W2.shape[1]

    xf = (np.asarray(x, np.float32).reshape(N, d)
          * np.asarray(seq, np.float32).reshape(N, 1))
    src = np.asarray(edges[0], np.int64)
    dst = np.asarray(edges[1], np.int64)
    kprof, perms, idx_all, mask_all = _schedule(src, dst, N)
    cfg = Cfg(N, d, H1, C1, D2, kprof)

    w1 = np.asarray(W1, np.float32)
    wsrc = np.einsum("khc,hc->kh", w1.reshape(d, H1, C1),
                     np.asarray(att_src1, np.float32))
    wdst = np.einsum("khc,hc->kh", w1.reshape(d, H1, C1),
                     np.asarray(att_dst1, np.float32))
    wpack1 = np.concatenate([w1, wsrc, wdst], axis=1).astype(np.float32)

    w2a = np.asarray(W2, np.float32)
    a2s = np.asarray(att_src2, np.float32).reshape(-1)
    a2d = np.asarray(att_dst2, np.float32).reshape(-1)
    c2_const = 0.0
    b2eff = np.asarray(b2, np.float32)
    w2ext = np.concatenate(
        [w2a, (w2a @ a2s)[:, None], (w2a @ a2d)[:, None]], axis=1
    ).astype(np.float32)

    b1r = np.tile(np.asarray(b1, np.float32)[None, :], (128, 1))
    b2effr = np.tile(b2eff[None, :], (128, 1))
    iota = np.tile(np.arange(128, dtype=np.float32)[None, :], (128, 1))
    iotac = np.arange(128, dtype=np.float32)[:, None].copy()

    in_maps = []
    for c in range(NCORES):
        xt_c = np.ascontiguousarray(xf[perms[c] + c * NSH].T)
        in_maps.append({
            "xt": xt_c, "wpack1": wpack1, "w2ext": w2ext,
            "b1r": b1r, "b2effr": b2effr, "iota": iota, "iotac": iotac,
            "eidx": idx_all[c], "emask": mask_all[c],
        })
    return cfg, c2_const, perms, in_maps


_CACHE = {}
LAST_RESULT = None


def kernel(**inputs) -> np.ndarray:
    from concourse.bass_utils import run_bass_kernel_spmd

    global LAST_RESULT
    x = np.asarray(inputs["x"])
    nb, ncn, d = x.shape
    cfg, c2_const, perms, in_maps = prepare(**{k: inputs[k] for k in (
        "x", "seq", "edges", "W1", "att_src1", "att_dst1", "b1",
        "W2", "att_src2", "att_dst2", "b2")})

    key = (cfg.N, cfg.D, cfg.H1, cfg.C1, cfg.D2, tuple(cfg.KPROF),
           round(c2_const, 10))
    if key not in _CACHE:
        _CACHE.clear()
        _CACHE[key] = build_program(cfg, c2_const)
    nc = _CACHE[key]

    res = run_bass_kernel_spmd(nc, in_maps, core_ids=list(range(NCORES)),
                               trace=False)
    LAST_RESULT = res
    full = np.zeros((cfg.N, cfg.D2), dtype=np.float32)
    for c in range(NCORES):
        full[perms[c] + c * NSH] = res.results[c]["out"]
    return full.reshape(nb, ncn, d).astype(np.float32)


# revision 17
# speedup vs baseline: 1.7853x; 1.7853x over previous
"""2-layer GAT (PyG GATConv semantics) on 8 Trainium2 NeuronCores via Bass/Tile.

Contract: kernel(**inputs) takes the FULL inputs of reference.setup_inputs()
and returns the FULL [16, 4096, 128] float32 output.

v2 design (dst-node sharding, degree-sorted blocks, dma_gather edge fetch):
- Core c owns dst nodes [c*8192, (c+1)*8192). Within a core, nodes are ranked
  by in-degree (self-loops excluded; they are folded analytically in the
  epilogue). Block b = ranks [128b, 128b+128); partition p holds the block's
  p-th node. Slot (b, p, j) = j-th in-edge of that node, padded per block to
  K[b] = max cross-core block degree (degree sorting makes padding ~5%).
- Node tables in DRAM, bf16, ONE physical layout shared by both layers
  (slice-major rank order), so a single int16 index array (phys(src)-32768,
  signed, table base mid-table) and a single pad mask drive both layers:
    t1 [N,128]: [h1(64) | asrc1(8) | adst1(8) | pad]     (256B rows)
    t2 [N,256]: [h2'(128) | asrc2'(1) | pad]             (512B rows)
- Phase A (sharded): t1 shard = xT @ [W1|wsrc1|wdst1], AllGather in 8 slices.
- Edge phase per block: one dma_gather (queue b%4, ~128*K[b] rows); softmax
  weights w = exp(lrelu(asrc[src]+adst[dst]+mask)) batched per block on
  DVE+ACT (denominator via ACT accum_out / reduce); rhs = gat*w in one fused
  DVE op; PSUM accumulation via identity-lhsT matmuls (one per 128-edge
  chunk). Self-loop terms w_self*h_self are added in the epilogue.
- L1 epilogue: y = acc/s + b1; (elu+1) fold: t2 stores h2' = (elu+1)@W2ext
  with W2ext = [W2 | W2@a2s | W2@a2d]; bias/logit constants folded into
  b2eff = b2 - colsum(W2) and adw2 = adst2' - c2. t2s rows AllGathered into
  t2 after every 8 blocks.
- L2 epilogue: out = acc2/s2 + b2eff, rows in rank order; host unpermutes.
"""

import os
import sys

import numpy as np

if "/opt/trn_rl_repo" not in sys.path:
    sys.path.insert(0, "/opt/trn_rl_repo")

import concourse.bass as bass
import concourse.bacc as bacc
import concourse.mybir as mybir
import concourse.tile as tile
from concourse.tile_rust import add_dep_helper

F32 = mybir.dt.float32
BF16 = mybir.dt.bfloat16
I16 = mybir.dt.int16
AOP = mybir.AluOpType
ACT = mybir.ActivationFunctionType

NEG_SLOPE = 0.2
NCORES = 8
BLK = 128
NSH = 8192
NBLK = NSH // BLK
NSLICE = 8
BPS = NBLK // NSLICE          # blocks per AG slice
SLN = NSH // NSLICE           # own rows per AG slice
T1W = 128                     # t1 cols (bf16): h1 64 | asrc 8 | adst 8 | pad
T2W = 256                     # t2 cols (bf16): h2' 128 | asrc2' 1 | pad
MASKVAL = -1e30
SCRATCH = 64 * 1024
USE_4D = os.environ.get("K4D", "1") == "1"


class Cfg:
    def __init__(self, n_nodes, d_in, h1, c1, d2, kprof):
        self.N = n_nodes
        self.D = d_in
        self.H1 = h1
        self.C1 = c1
        self.D1 = h1 * c1
        self.D2 = d2
        self.KPROF = list(kprof)          # per-block chunk counts (uniform)
        self.KSUM = int(sum(kprof))
        self.KOFF = np.concatenate([[0], np.cumsum(kprof)]).astype(int)


# ---------------------------------------------------------------------------
# host-side schedule
# ---------------------------------------------------------------------------
def _schedule(src, dst, N):
    """Degree-ranked per-core blocks; shared slot arrays for both layers.

    Returns (kprof, perm[c], idxw[c], maskw[c]) where idxw is the wrapped,
    replicated int16 index array [128, 8*KSUM] (values phys(src)-32768) and
    maskw the pad mask [128, KSUM] float32 (0 valid / MASKVAL pad).
    """
    core = dst >> 13
    perms = []
    degs = np.zeros((NCORES, NSH), dtype=np.int64)
    for c in range(NCORES):
        cnt = np.bincount(dst[core == c] - c * NSH, minlength=NSH)
        rank_to_node = np.argsort(-cnt, kind="stable")
        perms.append(rank_to_node)
        degs[c] = cnt[rank_to_node]
    # uniform per-block K profile (max over cores of block max degree)
    kprof = []
    for b in range(NBLK):
        kprof.append(int(max(1, degs[:, b * BLK:(b + 1) * BLK].max())))
    kprof = np.asarray(kprof, dtype=np.int64)
    ksum = int(kprof.sum())

    # phys mapping: node -> slice-major rank position (same for t1/t2)
    node_to_rank = np.zeros(N, dtype=np.int64)
    for c in range(NCORES):
        node_to_rank[perms[c] + c * NSH] = np.arange(NSH)
    s_of = node_to_rank >> 10
    phys = s_of * NSH + (np.arange(N, dtype=np.int64) >> 13) * 1024 \
        + (node_to_rank & 1023)

    order = np.argsort(dst, kind="stable")
    s_sorted, d_sorted = src[order], dst[order]
    starts = np.zeros(N + 1, dtype=np.int64)
    np.cumsum(np.bincount(d_sorted, minlength=N), out=starts[1:])

    # The gather ucode trims TRAILING NEGATIVE indices from each list, so the
    # last linear slot (p=127, j=kb-1) of every block must be >= 0. If node
    # 127's list is full and entirely negative, widen that block by one pad.
    for _ in range(3):
        koff = np.concatenate([[0], np.cumsum(kprof)])
        bump = np.zeros(NBLK, dtype=bool)
        for c in range(NCORES):
            for b in range(NBLK):
                kb = int(kprof[b])
                n = perms[c][b * BLK + 127] + c * NSH
                deg = int(starts[n + 1] - starts[n])
                if deg >= kb:
                    vals = phys[s_sorted[starts[n]:starts[n] + kb]] - 32768
                    if (vals < 0).all():
                        bump[b] = True
        if not bump.any():
            break
        kprof = kprof + bump.astype(np.int64)
    ksum = int(kprof.sum())
    koff = np.concatenate([[0], np.cumsum(kprof)])

    idx_all, mask_all = [], []
    for c in range(NCORES):
        lin = np.zeros((ksum, BLK), dtype=np.int16)      # [slotcol, p]
        msk = np.zeros((BLK, ksum), dtype=np.float32)
        msk[:] = MASKVAL
        for b in range(NBLK):
            kb = int(kprof[b])
            for p in range(BLK):
                n = perms[c][b * BLK + p] + c * NSH
                e0, e1 = int(starts[n]), int(starts[n + 1])
                deg = e1 - e0
                if deg:
                    vals = (phys[s_sorted[e0:e1]] - 32768).astype(np.int16)
                    if p == 127 and deg >= kb and vals[kb - 1] < 0:
                        nn = np.where(vals[:kb] >= 0)[0]
                        assert len(nn), "unfixable trailing-negative block"
                        vals = vals.copy()
                        vals[nn[0]], vals[kb - 1] = vals[kb - 1], vals[nn[0]]
                    lin[koff[b]:koff[b] + deg, p] = vals
                    msk[p, koff[b]:koff[b] + deg] = 0.0
        # wrap: linear i = j*128+p within each block -> [16, 8*K] per block
        iw = np.zeros((16, 8 * ksum), dtype=np.int16)
        for b in range(NBLK):
            kb = int(kprof[b])
            seg = lin[koff[b]:koff[b] + kb, :].reshape(-1)  # i = j*128+p
            ii = np.arange(kb * BLK)
            iw[ii % 16, 8 * koff[b] + ii // 16] = seg
        idx_all.append(np.tile(iw, (8, 1)))
        mask_all.append(msk)
    return kprof, perms, idx_all, mask_all


# ---------------------------------------------------------------------------
# device program
# ---------------------------------------------------------------------------
def build_program(cfg, c2_const):
    D, H1, C1, D1, D2 = cfg.D, cfg.H1, cfg.C1, cfg.D1, cfg.D2
    KPROF, KOFF, KSUM = cfg.KPROF, cfg.KOFF, cfg.KSUM
    KMAX = max(KPROF)
    N = cfg.N

    nc = bacc.Bacc("TRN2", target_bir_lowering=False, debug=False,
                   num_devices=NCORES, num_swdge_queues=4,
                   dynamic_dma_scratch_size=SCRATCH)

    xt = nc.dram_tensor("xt", [D, NSH], F32, kind="ExternalInput")
    wpack1 = nc.dram_tensor("wpack1", [D, D1 + 2 * H1], F32, kind="ExternalInput")
    w2ext = nc.dram_tensor("w2ext", [D1, D2 + 2], F32, kind="ExternalInput")
    b1r = nc.dram_tensor("b1r", [128, D1], F32, kind="ExternalInput")
    b2effr = nc.dram_tensor("b2effr", [128, D2], F32, kind="ExternalInput")
    iota = nc.dram_tensor("iota", [128, 128], F32, kind="ExternalInput")
    iotac = nc.dram_tensor("iotac", [128, 1], F32, kind="ExternalInput")
    eidx = nc.dram_tensor("eidx", [128, 8 * KSUM], I16, kind="ExternalInput")
    emask = nc.dram_tensor("emask", [128, KSUM], F32, kind="ExternalInput")
    out = nc.dram_tensor("out", [NSH, D2], F32, kind="ExternalOutput")

    dump = os.environ.get("KDUMP", "") == "1"
    t1s = nc.dram_tensor("t1s", [NSH, T1W], BF16, kind="Internal")
    t2s = nc.dram_tensor("t2s", [NSH, T2W], BF16, kind="Internal")
    if dump:
        t1d = nc.dram_tensor("t1d", [NSH, 80], F32, kind="ExternalOutput")
        t2d = nc.dram_tensor("t2d", [NSH, 130], F32, kind="ExternalOutput")
        yd = nc.dram_tensor("yd", [NSH, D1], F32, kind="ExternalOutput")
        sd = nc.dram_tensor("sd", [128, NBLK * H1], F32, kind="ExternalOutput")
        gd = nc.dram_tensor("gd", [128, KMAX * T1W], F32, kind="ExternalOutput")
    t1 = nc.dram_tensor("t1", [N, T1W], BF16, kind="Internal", addr_space="Shared")
    t2 = nc.dram_tensor("t2", [N, T2W], BF16, kind="Internal", addr_space="Shared")

    from concourse import library_config

    with tile.TileContext(nc) as tc:
        with tc.tile_pool(name="const", bufs=1) as cp:
            nc.gpsimd.load_library(library_config.mlp)
            con = {}
            for name, hndl in [("wpack1", wpack1), ("w2ext", w2ext),
                               ("b1r", b1r), ("b2effr", b2effr),
                               ("iota", iota), ("iotac", iotac)]:
                t = cp.tile(list(hndl.shape), hndl.dtype, tag=name)
                nc.sync.dma_start(out=t[:], in_=hndl[:])
                con[name] = t
            ident_bf = cp.tile([128, 128], BF16)
            nc.vector.tensor_tensor(
                out=ident_bf[:], in0=con["iotac"][:].to_broadcast([128, 128]),
                in1=con["iota"][:], op=AOP.is_equal)
            ident_f = cp.tile([128, 128], F32)
            nc.vector.tensor_tensor(
                out=ident_f[:], in0=con["iotac"][:].to_broadcast([128, 128]),
                in1=con["iota"][:], op=AOP.is_equal)
            con["ident_bf"] = ident_bf
            con["ident_f"] = ident_f
            # resident edge schedule
            eidx_t = cp.tile([128, 8 * KSUM], I16, tag="eidx")
            nc.sync.dma_start(out=eidx_t[:], in_=eidx[:])
            emask_t = cp.tile([128, KSUM], F32, tag="emask")
            nc.sync.dma_start(out=emask_t[:], in_=emask[:])
            con["eidx"] = eidx_t
            con["emask"] = emask_t

            # ---------------- phase A: t1 shard + AllGather ----------------
            ag1 = []
            with (tc.tile_pool(name="pa_ps", bufs=4, space="PSUM") as pps,
                  tc.tile_pool(name="pa_st", bufs=4) as pst):
                for t in range(NBLK):
                    ps = pps.tile([128, D1 + 2 * H1], F32, tag="ps")
                    xtile = pst.tile([128, 128], F32, tag="xtile")
                    nc.sync.dma_start(out=xtile[:], in_=xt[:, t * 128:(t + 1) * 128])
                    nc.tensor.matmul(out=ps[:], lhsT=xtile[:], rhs=con["wpack1"][:],
                                     start=True, stop=True)
                    stg = pst.tile([128, D1 + 2 * H1], BF16, tag="stg")
                    nc.scalar.copy(out=stg[:], in_=ps[:])
                    nc.sync.dma_start(
                        out=t1s[t * 128:(t + 1) * 128, 0:D1 + 2 * H1], in_=stg[:])
                    if dump:
                        stgf = pst.tile([128, D1 + 2 * H1], F32, tag="stgf")
                        nc.vector.tensor_copy(out=stgf[:], in_=stg[:])
                        nc.sync.dma_start(
                            out=t1d[t * 128:(t + 1) * 128, :], in_=stgf[:])
                    if (t + 1) % BPS == 0:
                        s = (t + 1) // BPS - 1
                        g = nc.gpsimd.collective_compute(
                            "AllGather", AOP.bypass,
                            replica_groups=[list(range(NCORES))],
                            ins=[t1s[s * SLN:(s + 1) * SLN, :]],
                            outs=[t1[s * SLN * NCORES:(s + 1) * SLN * NCORES, :]])
                        ag1.append(g)

            # own-shard L1 self data: [128p, NBLK, 80] (h1|asrc|adst)
            t1self = cp.tile([128, NBLK, D1 + 2 * H1], BF16, tag="t1self")
            src_ap = bass.AP(
                t1s[:].tensor, 0,
                [[T1W, 128], [BLK * T1W, NBLK], [1, D1 + 2 * H1]])
            nc.sync.dma_start(out=t1self[:], in_=src_ap)

            # batched L1 self weights: wself [128, NBLK*H1] f32
            wself1 = cp.tile([128, NBLK * H1], F32, tag="wself1")
            zs = cp.tile([128, NBLK * H1], F32, tag="zs")
            nc.vector.tensor_tensor(
                out=zs[:], in0=t1self[:, :, D1:D1 + H1],
                in1=t1self[:, :, D1 + H1:D1 + 2 * H1], op=AOP.add)
            nc.vector.scalar_tensor_tensor(
                out=zs[:], in0=zs[:], scalar=NEG_SLOPE, in1=zs[:],
                op0=AOP.mult, op1=AOP.max)
            nc.scalar.activation(out=wself1[:], in_=zs[:], func=ACT.Exp)

            adw2_all = cp.tile([128, NBLK], F32, tag="adw2")
            asrc2s_all = cp.tile([128, NBLK], F32, tag="asrc2s")

            # ---------------- L1 edge phase -------------------------------
            ag2 = []
            with (tc.tile_pool(name="e1_g", bufs=4) as pg,
                  tc.tile_pool(name="e1_w", bufs=2) as pw,
                  tc.tile_pool(name="e1_r", bufs=2) as pr,
                  tc.tile_pool(name="e1_acc", bufs=2, space="PSUM") as pacc,
                  tc.tile_pool(name="e1_tp", bufs=2, space="PSUM") as ptp,
                  tc.tile_pool(name="e1_h2", bufs=2, space="PSUM") as ph2,
                  tc.tile_pool(name="e1_ep", bufs=2) as pep):
                for b in range(NBLK):
                    kb = KPROF[b]
                    gat = pg.tile([128, KMAX, T1W], BF16, tag="gat")
                    gi = nc.gpsimd.dma_gather(
                        out_ap=gat[:, 0:kb, :], in_ap=t1[32768:, :],
                        idxs_ap=con["eidx"][:, 8 * KOFF[b]:8 * KOFF[b + 1]],
                        num_idxs=128 * kb, num_idxs_reg=128 * kb,
                        elem_size=T1W, queue_num=b % 4, single_packet=False)
                    for g in ag1:
                        add_dep_helper(gi.ins, g.ins, reason="t1 full-table read")

                    # w batch [128, kb*H1]
                    lg = pw.tile([128, KMAX * H1], F32, tag="lg")
                    adw = t1self[:, b:b + 1, D1 + H1:D1 + 2 * H1]
                    adw_bc = bass.AP(adw.tensor, adw.offset,
                                     [list(adw.ap[0]), [0, kb], [1, H1]])
                    nc.vector.tensor_tensor(
                        out=lg[:, 0:kb * H1].rearrange("p (k h) -> p k h", h=H1),
                        in0=gat[:, 0:kb, D1:D1 + H1], in1=adw_bc, op=AOP.add)
                    mslice = con["emask"][:, KOFF[b]:KOFF[b + 1]]
                    m_bc = bass.AP(mslice.tensor, mslice.offset,
                                   [list(mslice.ap[0]), list(mslice.ap[1]), [0, H1]])
                    nc.vector.tensor_tensor(
                        out=lg[:, 0:kb * H1].rearrange("p (k h) -> p k h", h=H1),
                        in0=lg[:, 0:kb * H1].rearrange("p (k h) -> p k h", h=H1),
                        in1=m_bc, op=AOP.add)
                    nc.vector.scalar_tensor_tensor(
                        out=lg[:, 0:kb * H1], in0=lg[:, 0:kb * H1],
                        scalar=NEG_SLOPE, in1=lg[:, 0:kb * H1],
                        op0=AOP.mult, op1=AOP.max)
                    w_t = pw.tile([128, KMAX * H1], F32, tag="w")
                    nc.scalar.activation(out=w_t[:, 0:kb * H1],
                                         in_=lg[:, 0:kb * H1], func=ACT.Exp)

                    # s[p,h] = sum_j w + wself
                    s_t = pep.tile([128, H1], F32, tag="s")
                    nc.vector.reduce_sum(
                        out=s_t[:],
                        in_=w_t[:, 0:kb * H1].rearrange("p (k h) -> p h k", h=H1),
                        axis=mybir.AxisListType.X)
                    nc.vector.tensor_add(
                        out=s_t[:], in0=s_t[:],
                        in1=wself1[:, b * H1:(b + 1) * H1])

                    # rhs = gat[:, :, 0:64] * w  (4D broadcast)
                    rhs = pr.tile([128, KMAX * D1], BF16, tag="rhs")
                    if USE_4D:
                        gv = gat[:, 0:kb, 0:D1].rearrange(
                            "p k (h c) -> p k h c", h=H1)
                        wv = bass.AP(
                            w_t[:].tensor, w_t[:].offset,
                            [list(w_t[:].ap[0]), [H1, kb], [1, H1], [0, C1]])
                        rv = rhs[:, 0:kb * D1].rearrange(
                            "p (k h c) -> p k h c", h=H1, c=C1)
                        nc.vector.tensor_tensor(out=rv, in0=gv, in1=wv, op=AOP.mult)
                    else:
                        for h in range(H1):
                            wcol = bass.AP(
                                w_t[:].tensor, w_t[:].offset + h,
                                [list(w_t[:].ap[0]), [H1, kb], [0, C1]])
                            nc.vector.tensor_tensor(
                                out=rhs[:, 0:kb * D1].rearrange(
                                    "p (k c) -> p k c", c=D1)[:, :, h * C1:(h + 1) * C1],
                                in0=gat[:, 0:kb, h * C1:(h + 1) * C1],
                                in1=wcol, op=AOP.mult)

                    acc = pacc.tile([128, D1], F32, tag="acc")
                    for j in range(kb):
                        nc.tensor.matmul(
                            out=acc[:], lhsT=con["ident_bf"][:],
                            rhs=rhs[:, j * D1:(j + 1) * D1],
                            start=(j == 0), stop=(j == kb - 1))

                    # ---- epilogue ----
                    vv = pep.tile([128, D1], F32, tag="vv")
                    wssl = wself1[:, b * H1:(b + 1) * H1]
                    wsv = bass.AP(wssl.tensor, wssl.offset,
                                  [list(wssl.ap[0]), [1, H1], [0, C1]])
                    h1self = t1self[:, b:b + 1, 0:D1]
                    h1v = bass.AP(h1self.tensor, h1self.offset,
                                  [list(h1self.ap[0]), [C1, H1], [1, C1]])
                    nc.vector.tensor_tensor(
                        out=vv[:].rearrange("p (h c) -> p h c", h=H1),
                        in0=h1v, in1=wsv, op=AOP.mult)
                    nc.vector.tensor_add(out=vv[:], in0=vv[:], in1=acc[:])
                    sinv = pep.tile([128, H1], F32, tag="sinv")
                    nc.vector.reciprocal(out=sinv[:], in_=s_t[:])
                    y = pep.tile([128, D1], F32, tag="y")
                    sinv_bc = bass.AP(
                        sinv[:].tensor, sinv[:].offset,
                        [list(sinv[:].ap[0]), [1, H1], [0, C1]])
                    nc.vector.tensor_tensor(
                        out=y[:].rearrange("p (h c) -> p h c", h=H1),
                        in0=vv[:].rearrange("p (h c) -> p h c", h=H1),
                        in1=sinv_bc, op=AOP.mult)
                    nc.vector.tensor_add(out=y[:], in0=y[:], in1=con["b1r"][:])
                    if dump:
                        nc.sync.dma_start(out=yd[b * BLK:(b + 1) * BLK, :],
                                          in_=y[:])
                        nc.sync.dma_start(out=sd[:, b * H1:(b + 1) * H1],
                                          in_=s_t[:])
                        if b == 0:
                            gdf = pep.tile([128, KMAX * T1W], F32, tag="gdf")
                            nc.vector.tensor_copy(
                                out=gdf[:],
                                in_=gat[:].rearrange("p k w -> p (k w)"))
                            nc.sync.dma_start(out=gd[:], in_=gdf[:])
                    tmin = pep.tile([128, D1], F32, tag="tmin")
                    nc.vector.scalar_tensor_tensor(
                        out=tmin[:], in0=y[:], scalar=0.0, in1=y[:],
                        op0=AOP.min, op1=AOP.min)
                    e_t = pep.tile([128, D1], F32, tag="e")
                    nc.scalar.activation(out=e_t[:], in_=tmin[:], func=ACT.Exp)
                    helu = pep.tile([128, D1], F32, tag="helu")
                    nc.vector.scalar_tensor_tensor(
                        out=helu[:], in0=y[:], scalar=0.0, in1=e_t[:],
                        op0=AOP.max, op1=AOP.add)
                    nc.vector.scalar_tensor_tensor(
                        out=helu[:], in0=helu[:], scalar=-1.0, in1=helu[:],
                        op0=AOP.add, op1=AOP.min)
                    htp = ptp.tile([D1, 128], F32, tag="htp")
                    nc.tensor.transpose(out=htp[:], in_=helu[:],
                                        identity=con["ident_f"][:])
                    hts = pep.tile([D1, 128], F32, tag="hts")
                    nc.scalar.copy(out=hts[:], in_=htp[:])
                    h2p = ph2.tile([128, D2 + 2], F32, tag="h2p")
                    nc.tensor.matmul(out=h2p[:], lhsT=hts[:], rhs=con["w2ext"][:],
                                     start=True, stop=True)
                    # stash asrc2', adw2 = adst2' - c2
                    nc.scalar.activation(
                        out=asrc2s_all[:, b:b + 1], in_=h2p[:, D2:D2 + 1],
                        func=ACT.Copy)
                    nc.scalar.activation(
                        out=adw2_all[:, b:b + 1], in_=h2p[:, D2 + 1:D2 + 2],
                        func=ACT.Copy)
                    stg2 = pep.tile([128, D2 + 1], BF16, tag="stg2")
                    nc.scalar.copy(out=stg2[:], in_=h2p[:, 0:D2 + 1])
                    nc.sync.dma_start(
                        out=t2s[b * BLK:(b + 1) * BLK, 0:D2 + 1], in_=stg2[:])
                    if dump:
                        stg2f = pep.tile([128, D2 + 2], F32, tag="stg2f")
                        nc.vector.tensor_copy(out=stg2f[:], in_=h2p[:])
                        nc.sync.dma_start(
                            out=t2d[b * BLK:(b + 1) * BLK, 0:D2 + 1],
                            in_=stg2f[:, 0:D2 + 1])

                    if (b + 1) % BPS == 0:
                        s = (b + 1) // BPS - 1
                        g = nc.gpsimd.collective_compute(
                            "AllGather", AOP.bypass,
                            replica_groups=[list(range(NCORES))],
                            ins=[t2s[s * SLN:(s + 1) * SLN, :]],
                            outs=[t2[s * SLN * NCORES:(s + 1) * SLN * NCORES, :]])
                        ag2.append(g)

            # own-shard L2 self data
            t2self = cp.tile([128, NBLK, D2], BF16, tag="t2self")
            src2_ap = bass.AP(
                t2s[:].tensor, 0, [[T2W, 128], [BLK * T2W, NBLK], [1, D2]])
            d2s = nc.sync.dma_start(out=t2self[:], in_=src2_ap)
            wself2 = cp.tile([128, NBLK], F32, tag="wself2")
            zs2 = cp.tile([128, NBLK], F32, tag="zs2")
            nc.vector.tensor_tensor(out=zs2[:], in0=asrc2s_all[:],
                                    in1=adw2_all[:], op=AOP.add)
            nc.vector.scalar_tensor_tensor(
                out=zs2[:], in0=zs2[:], scalar=NEG_SLOPE, in1=zs2[:],
                op0=AOP.mult, op1=AOP.max)
            nc.scalar.activation(out=wself2[:], in_=zs2[:], func=ACT.Exp)

            # ---------------- L2 edge phase -------------------------------
            with (tc.tile_pool(name="e2_g", bufs=4) as pg2,
                  tc.tile_pool(name="e2_w", bufs=2) as pw2,
                  tc.tile_pool(name="e2_r", bufs=2) as pr2,
                  tc.tile_pool(name="e2_acc", bufs=2, space="PSUM") as pacc2,
                  tc.tile_pool(name="e2_ep", bufs=2) as pep2):
                for b in range(NBLK):
                    kb = KPROF[b]
                    gat2 = pg2.tile([128, KMAX, T2W], BF16, tag="gat2")
                    gi = nc.gpsimd.dma_gather(
                        out_ap=gat2[:, 0:kb, :], in_ap=t2[32768:, :],
                        idxs_ap=con["eidx"][:, 8 * KOFF[b]:8 * KOFF[b + 1]],
                        num_idxs=128 * kb, num_idxs_reg=128 * kb,
                        elem_size=T2W, queue_num=b % 4, single_packet=False)
                    for g in ag2:
                        add_dep_helper(gi.ins, g.ins, reason="t2 full-table read")

                    lg2 = pw2.tile([128, KMAX], F32, tag="lg2")
                    asr = gat2[:, 0:kb, D2:D2 + 1]
                    asr2d = bass.AP(asr.tensor, asr.offset,
                                    [list(asr.ap[0]), [T2W, kb]])
                    nc.vector.tensor_tensor(
                        out=lg2[:, 0:kb], in0=asr2d,
                        in1=adw2_all[:, b:b + 1].to_broadcast([128, kb]),
                        op=AOP.add)
                    nc.vector.tensor_add(
                        out=lg2[:, 0:kb], in0=lg2[:, 0:kb],
                        in1=con["emask"][:, KOFF[b]:KOFF[b + 1]])
                    nc.vector.scalar_tensor_tensor(
                        out=lg2[:, 0:kb], in0=lg2[:, 0:kb], scalar=NEG_SLOPE,
                        in1=lg2[:, 0:kb], op0=AOP.mult, op1=AOP.max)
                    w2t = pw2.tile([128, KMAX], F32, tag="w2t")
                    s2 = pep2.tile([128, 1], F32, tag="s2")
                    nc.scalar.activation(out=w2t[:, 0:kb], in_=lg2[:, 0:kb],
                                         func=ACT.Exp, accum_out=s2[:])
                    nc.vector.tensor_add(out=s2[:], in0=s2[:],
                                         in1=wself2[:, b:b + 1])

                    rhs2 = pr2.tile([128, KMAX * D2], BF16, tag="rhs2")
                    if b % 2 == 0:
                        wv = bass.AP(w2t[:].tensor, w2t[:].offset,
                                     [list(w2t[:].ap[0]), [1, kb], [0, D2]])
                        nc.vector.tensor_tensor(
                            out=rhs2[:, 0:kb * D2].rearrange(
                                "p (k d) -> p k d", d=D2),
                            in0=gat2[:, 0:kb, 0:D2], in1=wv, op=AOP.mult)
                    else:
                        for j in range(kb):
                            gsl = gat2[:, j:j + 1, 0:D2]
                            g2d = bass.AP(gsl.tensor, gsl.offset,
                                          [list(gsl.ap[0]), [1, D2]])
                            nc.scalar.activation(
                                out=rhs2[:, j * D2:(j + 1) * D2], in_=g2d,
                                func=ACT.Copy, scale=w2t[:, j:j + 1])

                    acc2 = pacc2.tile([128, D2], F32, tag="acc2")
                    for j in range(kb):
                        nc.tensor.matmul(
                            out=acc2[:], lhsT=con["ident_bf"][:],
                            rhs=rhs2[:, j * D2:(j + 1) * D2],
                            start=(j == 0), stop=(j == kb - 1))

                    # epilogue: out = (acc2 + wself2*h2self)/s2 + b2eff
                    av = pep2.tile([128, D2], F32, tag="av")
                    ws_bc = wself2[:, b:b + 1].to_broadcast([128, D2])
                    nc.vector.tensor_tensor(out=av[:], in0=t2self[:, b, :],
                                            in1=ws_bc, op=AOP.mult)
                    nc.vector.tensor_add(out=av[:], in0=av[:], in1=acc2[:])
                    sinv2 = pep2.tile([128, 1], F32, tag="sinv2")
                    nc.vector.reciprocal(out=sinv2[:], in_=s2[:])
                    o1 = pep2.tile([128, D2], F32, tag="o1")
                    nc.scalar.activation(out=o1[:], in_=av[:], func=ACT.Copy,
                                         scale=sinv2[:])
                    o2 = pep2.tile([128, D2], F32, tag="o2")
                    nc.vector.tensor_add(out=o2[:], in0=o1[:], in1=con["b2effr"][:])
                    nc.sync.dma_start(out=out[b * BLK:(b + 1) * BLK, :], in_=o2[:])

    nc.compile()
    return nc


# ---------------------------------------------------------------------------
# host glue
# ---------------------------------------------------------------------------
def prepare(x, seq, edges, W1, att_src1, att_dst1, b1, W2, att_src2,
            att_dst2, b2):
    import ml_dtypes

    nb, ncn, d = x.shape
    N = nb * ncn
    H1, C1 = att_src1.shape
    D1 = H1 * C1
    D2 = W2.shape[1]

    xf = (np.asarray(x, np.float32).reshape(N, d)
          * np.asarray(seq, np.float32).reshape(N, 1))
    src = np.asarray(edges[0], np.int64)
    dst = np.asarray(edges[1], np.int64)
    kprof, perms, idx_all, mask_all = _schedule(src, dst, N)
    cfg = Cfg(N, d, H1, C1, D2, kprof)

    w1 = np.asarray(W1, np.float32)
    wsrc = np.einsum("khc,hc->kh", w1.reshape(d, H1, C1),
                     np.asarray(att_src1, np.float32))
    wdst = np.einsum("khc,hc->kh", w1.reshape(d, H1, C1),
                     np.asarray(att_dst1, np.float32))
    wpack1 = np.concatenate([w1, wsrc, wdst], axis=1).astype(np.float32)

    w2a = np.asarray(W2, np.float32)
    a2s = np.asarray(att_src2, np.float32).reshape(-1)
    a2d = np.asarray(att_dst2, np.float32).reshape(-1)
    c2_const = 0.0
    b2eff = np.asarray(b2, np.float32)
    w2ext = np.concatenate(
        [w2a, (w2a @ a2s)[:, None], (w2a @ a2d)[:, None]], axis=1
    ).astype(np.float32)

    b1r = np.tile(np.asarray(b1, np.float32)[None, :], (128, 1))
    b2effr = np.tile(b2eff[None, :], (128, 1))
    iota = np.tile(np.arange(128, dtype=np.float32)[None, :], (128, 1))
    iotac = np.arange(128, dtype=np.float32)[:, None].copy()

    in_maps = []
    for c in range(NCORES):
        xt_c = np.ascontiguousarray(xf[perms[c] + c * NSH].T)
        in_maps.append({
            "xt": xt_c, "wpack1": wpack1, "w2ext": w2ext,
            "b1r": b1r, "b2effr": b2effr, "iota": iota, "iotac": iotac,
            "eidx": idx_all[c], "emask": mask_all[c],
        })
    return cfg, c2_const, perms, in_maps


_CACHE = {}
LAST_RESULT = None


def kernel(**inputs) -> np.ndarray:
    from concourse.bass_utils import run_bass_kernel_spmd

    global LAST_RESULT
    x = np.asarray(inputs["x"])
    nb, ncn, d = x.shape
    cfg, c2_const, perms, in_maps = prepare(**{k: inputs[k] for k in (
        "x", "seq", "edges", "W1", "att_src1", "att_dst1", "b1",
        "W2", "att_src2", "att_dst2", "b2")})

    key = (cfg.N, cfg.D, cfg.H1, cfg.C1, cfg.D2, tuple(cfg.KPROF),
           round(c2_const, 10))
    if key not in _CACHE:
        _CACHE.clear()
        _CACHE[key] = build_program(cfg, c2_const)
    nc = _CACHE[key]

    res = run_bass_kernel_spmd(nc, in_maps, core_ids=list(range(NCORES)),
                               trace=False)
    LAST_RESULT = res
    full = np.zeros((cfg.N, cfg.D2), dtype=np.float32)
    for c in range(NCORES):
        full[perms[c] + c * NSH] = res.results[c]["out"]
    return full.reshape(nb, ncn, d).astype(np.float32)
